# revision 8
# baseline (speedup 1.0000x reference)
"""Context-parallel causal attention block on 8 Trainium2 NeuronCores.

Strategy: tensor-parallel split-heads. Each core c computes Q/K/V projections
for its 2 heads (of 16) over all tokens with host-sliced weights, runs causal
attention locally (feature-major layouts, no transposes), then one on-device
AllToAll re-shards from head-parallel to token-parallel, and each core runs the
output projection for its 512-token row slice. Host concatenates row slices.

Schedule (v2): projections interleave with head-0 attention only; AllToAll#0
fires right after the last head-0 unit and is covered by all head-1 attention;
AllToAll#1 is covered by the h0-feature half of the output projection (2-pass
oproj with an fp32 SBUF accumulator). Softmax row-sums are computed with one
ones-matmul per 4-block group (DVE pre-adds the exp tiles), and diagonal
blocks stream only their causally-active columns.

Matmul operands are bf16 (fp32 matmuls are two-pass / half-rate on TRN2's PE);
all accumulation stays fp32 in PSUM, softmax runs on fp32 scores.
"""
import sys

sys.path.insert(0, "/opt/trn_rl_repo")

import ml_dtypes
import numpy as np

import concourse.bass as bass
import concourse.tile as tile
from concourse import bacc, mybir
from concourse.bass_utils import run_bass_kernel_spmd

FP = mybir.dt.float32
BF = mybir.dt.bfloat16
NPBF = ml_dtypes.bfloat16
N_CORES = 8
B, S, D, H, DH = 2, 2048, 2048, 16, 128
T = B * S            # 4096 flattened tokens, b-major
KK = D // 128        # 16 contraction k-tiles
NSTRIP = T // 512    # 8 token strips of 512
ROWS = T // N_CORES  # 512 output rows per core
NEG = -1.0e30


def build_nc() -> bacc.Bacc:
    nc = bacc.Bacc("TRN2", target_bir_lowering=False, debug=False, num_devices=N_CORES)

    xt = nc.dram_tensor("xt", [128, KK, T], BF, kind="ExternalInput")
    wq = nc.dram_tensor("wq", [128, KK, 256], BF, kind="ExternalInput")
    wk = nc.dram_tensor("wk", [128, KK, 256], BF, kind="ExternalInput")
    wv = nc.dram_tensor("wv", [128, KK, 256], BF, kind="ExternalInput")
    # wo laid out host-side as [p, h, i, d]: feature tile (2i+h), out col d
    wo = nc.dram_tensor("wo", [128, 2, 8, D], BF, kind="ExternalInput")
    out_t = nc.dram_tensor("out_t", [D, ROWS], FP, kind="ExternalOutput")

    with tile.TileContext(nc) as tc:
        with (
            tc.tile_pool(name="dram", bufs=1, space="DRAM") as dram,
            tc.tile_pool(name="consts", bufs=1) as consts,
            tc.tile_pool(name="persist", bufs=1) as persist,
        ):
            a2a_in = [dram.tile([N_CORES, 128, 512], BF, name=f"a2a_in{h}") for h in range(2)]
            a2a_out = [dram.tile([N_CORES, 128, 512], BF, name=f"a2a_out{h}") for h in range(2)]
            wu_in = dram.tile([N_CORES, 1, 8], BF, name="wu_in")
            wu_out = dram.tile([N_CORES, 1, 8], BF, name="wu_out")

            ones = consts.tile([128, 1], BF)
            nc.gpsimd.memset(ones[:], 1.0)
            wu_sb = consts.tile([1, 64], BF)
            nc.gpsimd.memset(wu_sb[:], 0.0)
            nc.sync.dma_start(wu_in[:].rearrange("i o t -> o (i t)"), wu_sb[:])
            # warm up the collective stream so the real AllToAlls start fast
            nc.gpsimd.collective_compute(
                "AllToAll", mybir.AluOpType.bypass,
                replica_groups=[list(range(N_CORES))],
                ins=[wu_in[:].opt()],
                outs=[wu_out[:].opt()],
            )
            # additive causal masks for the 4 diagonal offsets:
            # masks[p, i, q] = 0 if q >= p + i*128 else NEG
            masks = consts.tile([128, 4, 512], FP)
            nc.gpsimd.memset(masks[:], 0.0)
            for i in range(4):
                nc.gpsimd.affine_select(
                    out=masks[:, i, :],
                    in_=masks[:, i, :],
                    compare_op=mybir.AluOpType.is_ge,
                    fill=NEG,
                    base=-(i * 128),
                    pattern=[[1, 512]],
                    channel_multiplier=-1,
                )

            qT = persist.tile([128, 2, T], BF)       # [dh, hl, t]
            kT = persist.tile([128, 2, T], BF)
            v_sb = persist.tile([128, 32, 256], BF)  # [t%128, t//128, head_feat]
            acc0 = persist.tile([128, KK, 512], FP)  # oproj h0-pass partials
            ot_sb = [persist.tile([128, 8, 512], BF, name=f"ot_sb{h}") for h in range(2)]

            with (
                tc.tile_pool(name="wpool", bufs=1) as wpool,
                tc.tile_pool(name="xtp", bufs=6) as xtp,
                tc.tile_pool(name="ps1", bufs=2, space="PSUM") as ps1,
                tc.tile_pool(name="expp", bufs=5) as expp,
                tc.tile_pool(name="gp", bufs=2) as gp,
                tc.tile_pool(name="smallp", bufs=2) as smallp,
                tc.tile_pool(name="otp", bufs=3) as otp,
                tc.tile_pool(name="wop", bufs=6) as wop,
                tc.tile_pool(name="psT", bufs=2, space="PSUM") as psT,
                tc.tile_pool(name="psA", bufs=1, space="PSUM") as psA,
                tc.tile_pool(name="psS", bufs=1, space="PSUM") as psS,
            ):
                wq_sb = wpool.tile([128, KK, 256], BF)
                wk_sb = wpool.tile([128, KK, 256], BF)
                wv_sb = wpool.tile([128, KK, 256], BF)
                nc.sync.dma_start(wq_sb[:], wq[:])

                # ---- Phase 1+2: projections + head-0 attention, interleaved ----
                for b in range(B):
                    for s in range(4):
                        strip = b * 4 + s
                        t0 = strip * 512
                        xq = []
                        for qtr in range(4):
                            xtile = xtp.tile([128, 4, 512], BF, tag="xt")
                            nc.sync.dma_start(
                                xtile[:],
                                xt[:, qtr * 4 : (qtr + 1) * 4, t0 : t0 + 512],
                            )
                            xq.append(xtile)
                        if b == 0 and s == 0:
                            # k/v weights load behind strip-0 x so the q-pass
                            # starts as early as possible
                            nc.sync.dma_start(wk_sb[:], wk[:])
                            nc.sync.dma_start(wv_sb[:], wv[:])

                        # pass A: q for both heads (2 banks)
                        pa = [ps1.tile([128, 512], FP, tag=f"p1{j}", name=f"pa{j}")
                              for j in range(2)]
                        for kk in range(KK):
                            xsl = xq[kk // 4][:, kk % 4, :]
                            st, sp = kk == 0, kk == KK - 1
                            nc.tensor.matmul(pa[0][:], wq_sb[:, kk, 0:128], xsl, start=st, stop=sp)
                            nc.tensor.matmul(pa[1][:], wq_sb[:, kk, 128:256], xsl, start=st, stop=sp)
                        for hl in range(2):
                            nc.scalar.copy(qT[:, hl, t0 : t0 + 512], pa[hl][:])
                        # pass B: k for both heads
                        pb = [ps1.tile([128, 512], FP, tag=f"p1{j}", name=f"pb{j}")
                              for j in range(2)]
                        for kk in range(KK):
                            xsl = xq[kk // 4][:, kk % 4, :]
                            st, sp = kk == 0, kk == KK - 1
                            nc.tensor.matmul(pb[0][:], wk_sb[:, kk, 0:128], xsl, start=st, stop=sp)
                            nc.tensor.matmul(pb[1][:], wk_sb[:, kk, 128:256], xsl, start=st, stop=sp)
                        for hl in range(2):
                            nc.scalar.copy(kT[:, hl, t0 : t0 + 512], pb[hl][:])
                        # pass C/D: v in two tt sub-passes (2 banks each)
                        for half in range(2):
                            pv = [ps1.tile([128, 256], FP, tag=f"p1{j}", name=f"pv{j}")
                                  for j in range(2)]
                            for kk in range(KK):
                                xsl = xq[kk // 4][:, kk % 4, :]
                                st, sp = kk == 0, kk == KK - 1
                                for jj in range(2):
                                    tt = half * 2 + jj
                                    nc.tensor.matmul(
                                        pv[jj][:],
                                        xsl[:, tt * 128 : (tt + 1) * 128],
                                        wv_sb[:, kk, :],
                                        start=st,
                                        stop=sp,
                                    )
                            for jj in range(2):
                                nc.vector.tensor_copy(
                                    v_sb[:, strip * 4 + half * 2 + jj, :], pv[jj][:]
                                )

                        attention_unit(tc, nc, a2a_in, qT, kT, v_sb, ones, masks,
                                       expp, gp, smallp, otp, psT, psA, psS, b, 0, s)

                nc.gpsimd.collective_compute(
                    "AllToAll", mybir.AluOpType.bypass,
                    replica_groups=[list(range(N_CORES))],
                    ins=[a2a_in[0][:].opt()],
                    outs=[a2a_out[0][:].opt()],
                )
                # gpsimd queue: keeps the a2a-gated rearrange off the sync
                # queue so per-unit slot DMAs and wo loads never stall on it
                nc.gpsimd.dma_start(
                    ot_sb[0][:], a2a_out[0][:].rearrange("i f t -> f i t")
                )

                # ---- Phase 3: head-1 attention (covers AllToAll#0) ----
                for b in range(B):
                    for s in range(4):
                        attention_unit(tc, nc, a2a_in, qT, kT, v_sb, ones, masks,
                                       expp, gp, smallp, otp, psT, psA, psS, b, 1, s)

                nc.gpsimd.collective_compute(
                    "AllToAll", mybir.AluOpType.bypass,
                    replica_groups=[list(range(N_CORES))],
                    ins=[a2a_in[1][:].opt()],
                    outs=[a2a_out[1][:].opt()],
                )
                nc.gpsimd.dma_start(
                    ot_sb[1][:], a2a_out[1][:].rearrange("i f t -> f i t")
                )

                # ---- Phase 4: output projection, two passes ----
                # pass 0 (h0 features) overlaps AllToAll#1; pass 1 adds h1.
                for h in range(2):
                    for dd in range(KK):
                        wod = wop.tile([128, 8, 128], BF, tag="wod")
                        nc.sync.dma_start(wod[:], wo[:, h, :, dd * 128 : (dd + 1) * 128])
                        op = ps1.tile([128, 512], FP, tag="p10", name=f"op{h}_{dd}")
                        for i in range(8):
                            nc.tensor.matmul(
                                op[:],
                                wod[:, i, :],
                                ot_sb[h][:, i, :],
                                start=(i == 0),
                                stop=(i == 7),
                            )
                        if h == 0:
                            nc.scalar.copy(acc0[:, dd, :], op[:])
                        else:
                            ob = otp.tile([128, 512], FP, tag="ob")
                            nc.vector.tensor_add(ob[:], op[:], acc0[:, dd, :])
                            nc.sync.dma_start(out_t[dd * 128 : (dd + 1) * 128, :], ob[:])

    nc.compile()
    return nc


def attention_unit(tc, nc, a2a_in, qT, kT, v_sb, ones, masks,
                   expp, gp, smallp, otp, psT, psA, psS, b, hl, s):
    q0 = b * S + s * 512
    qts = qT[:, hl, q0 : q0 + 512]
    avp = psA.tile([128, 512], FP, tag="av")
    smp = psS.tile([1, 512], FP, tag="sm")
    nk = 4 * (s + 1)
    ngroups = s + 1
    for g in range(ngroups):
        diag = g == s
        exs = []
        for j in range(4):
            ki = g * 4 + j
            lo = j * 128 if diag else 0  # causally-active columns start here
            stp = psT.tile([128, 512], FP, tag="st")
            nc.tensor.matmul(
                stp[:, lo:],
                kT[:, hl, b * S + ki * 128 : b * S + (ki + 1) * 128],
                qts[:, lo:],
                start=True,
                stop=True,
            )
            if diag:
                nc.vector.tensor_add(stp[:, lo:], stp[:, lo:], masks[:, j, lo:])
            ex = expp.tile([128, 512], BF, tag="ex")
            nc.scalar.activation(ex[:, lo:], stp[:, lo:], mybir.ActivationFunctionType.Exp)
            nc.tensor.matmul(
                avp[:, lo:],
                v_sb[:, b * 16 + ki, hl * 128 : (hl + 1) * 128],
                ex[:, lo:],
                start=(ki == 0),
                stop=(ki == nk - 1),
                skip_group_check=True,
            )
            exs.append(ex)
        # per-group exp-sum: DVE folds 4 tiles, one ones-matmul per group
        gacc = gp.tile([128, 512], BF, tag="g")
        if diag:
            nc.vector.tensor_copy(gacc[:], exs[0][:])
            for j in range(1, 4):
                lo = j * 128
                nc.vector.tensor_add(gacc[:, lo:], gacc[:, lo:], exs[j][:, lo:])
        else:
            t01 = gp.tile([128, 512], BF, tag="t01")
            nc.vector.tensor_add(t01[:], exs[0][:], exs[1][:])
            nc.vector.tensor_add(gacc[:], exs[2][:], exs[3][:])
            nc.vector.tensor_add(gacc[:], gacc[:], t01[:])
        nc.tensor.matmul(smp[:], ones[:], gacc[:],
                         start=(g == 0), stop=(g == ngroups - 1))
    sums_sb = smallp.tile([1, 512], FP, tag="sums")
    nc.scalar.copy(sums_sb[:], smp[:])
    sbc = smallp.tile([128, 512], FP, tag="sbc")
    nc.gpsimd.partition_broadcast(sbc[:], sums_sb[:])
    rbc = smallp.tile([128, 512], FP, tag="rbc")
    nc.vector.reciprocal_approx_fast(rbc[:], sbc[:])
    ot = otp.tile([128, 512], BF, tag="ot")
    nc.vector.tensor_mul(ot[:], avp[:], rbc[:])
    j = b * 4 + s
    nc.sync.dma_start(a2a_in[hl][j, :, :], ot[:])


_NC_CACHE = {}


def _get_nc():
    if "nc" not in _NC_CACHE:
        _NC_CACHE["nc"] = build_nc()
    return _NC_CACHE["nc"]


def _make_in_maps(x, wq, wk, wv, wo):
    x = np.ascontiguousarray(np.asarray(x, dtype=np.float32))
    wq = np.asarray(wq, dtype=np.float32)
    wk = np.asarray(wk, dtype=np.float32)
    wv = np.asarray(wv, dtype=np.float32)
    wo = np.asarray(wo, dtype=np.float32)

    x_flat = x.reshape(T, D)
    # xt[p, kk, t] = x_flat[t, kk*128+p]
    xt_host = np.ascontiguousarray(
        x_flat.T.reshape(KK, 128, T).transpose(1, 0, 2)
    ).astype(NPBF)
    # wo_dev[p, h, i, d] = wo[d, (2i+h)*128+p]
    wo_host = np.ascontiguousarray(
        wo.T.reshape(8, 2, 128, D).transpose(2, 1, 0, 3)
    ).astype(NPBF)
    scale = 1.0 / np.sqrt(np.float32(DH))

    in_maps = []
    for c in range(N_CORES):
        sl = slice(c * 256, (c + 1) * 256)

        def wslice(w, scaled=False):
            wc = w[sl, :].T  # [D, 256]
            if scaled:
                wc = wc * scale
            return np.ascontiguousarray(
                wc.reshape(KK, 128, 256).transpose(1, 0, 2)
            ).astype(NPBF)

        in_maps.append(
            {
                "xt": xt_host,
                "wq": wslice(wq, scaled=True),
                "wk": wslice(wk),
                "wv": wslice(wv),
                "wo": wo_host,
            }
        )
    return in_maps


def _run(x, wq, wk, wv, wo, trace=False):
    nc = _get_nc()
    in_maps = _make_in_maps(x, wq, wk, wv, wo)
    res = run_bass_kernel_spmd(nc, in_maps, list(range(N_CORES)), trace=trace)
    rows = [res.results[c]["out_t"].T for c in range(N_CORES)]  # [512, D] each
    out = np.concatenate(rows, axis=0).reshape(B, S, D)
    return out, res


def kernel(x, wq, wk, wv, wo):
    out, _ = _run(x, wq, wk, wv, wo, trace=False)
    return out


# revision 13
# speedup vs baseline: 1.0387x; 1.0387x over previous
"""Context-parallel causal attention block on 8 Trainium2 NeuronCores.

Strategy: tensor-parallel split-heads. Each core c computes Q/K/V projections
for its 2 heads (of 16) over all tokens with host-sliced weights, runs causal
attention locally (feature-major layouts, no transposes), then one on-device
AllToAll re-shards from head-parallel to token-parallel, and each core runs the
output projection for its 512-token row slice. Host concatenates row slices.

Schedule (v2): projections interleave with head-0 attention only; AllToAll#0
fires right after the last head-0 unit and is covered by all head-1 attention;
AllToAll#1 is covered by the h0-feature half of the output projection (2-pass
oproj with an fp32 SBUF accumulator). Softmax row-sums are computed with one
ones-matmul per 4-block group (DVE pre-adds the exp tiles), and diagonal
blocks stream only their causally-active columns.

Matmul operands are bf16 (fp32 matmuls are two-pass / half-rate on TRN2's PE);
all accumulation stays fp32 in PSUM, softmax runs on fp32 scores.
"""
import sys

sys.path.insert(0, "/opt/trn_rl_repo")

import ml_dtypes
import numpy as np

import concourse.bass as bass
import concourse.tile as tile
from concourse import bacc, mybir
from concourse.bass_utils import run_bass_kernel_spmd

FP = mybir.dt.float32
BF = mybir.dt.bfloat16
NPBF = ml_dtypes.bfloat16
N_CORES = 8
B, S, D, H, DH = 2, 2048, 2048, 16, 128
T = B * S            # 4096 flattened tokens, b-major
KK = D // 128        # 16 contraction k-tiles
NSTRIP = T // 512    # 8 token strips of 512
ROWS = T // N_CORES  # 512 output rows per core
NEG = -1.0e30


def build_nc() -> bacc.Bacc:
    nc = bacc.Bacc("TRN2", target_bir_lowering=False, debug=False, num_devices=N_CORES)

    xt = nc.dram_tensor("xt", [128, KK, T], BF, kind="ExternalInput")
    wq = nc.dram_tensor("wq", [128, KK, 256], BF, kind="ExternalInput")
    wk = nc.dram_tensor("wk", [128, KK, 256], BF, kind="ExternalInput")
    wv = nc.dram_tensor("wv", [128, KK, 256], BF, kind="ExternalInput")
    # wo laid out host-side as [p, h, i, d]: feature tile (2i+h), out col d
    wo = nc.dram_tensor("wo", [128, 2, 8, D], BF, kind="ExternalInput")
    out_t = nc.dram_tensor("out_t", [D, ROWS], FP, kind="ExternalOutput")

    with tile.TileContext(nc) as tc:
        with (
            tc.tile_pool(name="dram", bufs=1, space="DRAM") as dram,
            tc.tile_pool(name="consts", bufs=1) as consts,
            tc.tile_pool(name="persist", bufs=1) as persist,
        ):
            a2a_in = [dram.tile([N_CORES, 128, 512], BF, name=f"a2a_in{h}") for h in range(2)]
            a2a_out = [dram.tile([N_CORES, 128, 512], BF, name=f"a2a_out{h}") for h in range(2)]
            wu_in = dram.tile([N_CORES, 1, 8], BF, name="wu_in")
            wu_out = dram.tile([N_CORES, 1, 8], BF, name="wu_out")

            ones = consts.tile([128, 1], BF)
            nc.gpsimd.memset(ones[:], 1.0)
            wu_sb = consts.tile([1, 64], BF)
            nc.gpsimd.memset(wu_sb[:], 0.0)
            nc.sync.dma_start(wu_in[:].rearrange("i o t -> o (i t)"), wu_sb[:])
            # warm up the collective stream so the real AllToAlls start fast
            nc.gpsimd.collective_compute(
                "AllToAll", mybir.AluOpType.bypass,
                replica_groups=[list(range(N_CORES))],
                ins=[wu_in[:].opt()],
                outs=[wu_out[:].opt()],
            )
            # additive causal masks for the 4 diagonal offsets:
            # masks[p, i, q] = 0 if q >= p + i*128 else NEG
            masks = consts.tile([128, 4, 512], FP)
            nc.gpsimd.memset(masks[:], 0.0)
            for i in range(4):
                nc.gpsimd.affine_select(
                    out=masks[:, i, :],
                    in_=masks[:, i, :],
                    compare_op=mybir.AluOpType.is_ge,
                    fill=NEG,
                    base=-(i * 128),
                    pattern=[[1, 512]],
                    channel_multiplier=-1,
                )

            qT = persist.tile([128, 2, T], BF)       # [dh, hl, t]
            kT = persist.tile([128, 2, T], BF)
            v_sb = persist.tile([128, 32, 256], BF)  # [t%128, t//128, head_feat]
            acc0 = persist.tile([128, KK, 512], FP)  # oproj h0-pass partials
            ot_sb = [persist.tile([128, 8, 512], BF, name=f"ot_sb{h}") for h in range(2)]

            with (
                tc.tile_pool(name="wpool", bufs=1) as wpool,
                tc.tile_pool(name="xtp", bufs=6) as xtp,
                tc.tile_pool(name="ps1", bufs=2, space="PSUM") as ps1,
                tc.tile_pool(name="expp", bufs=5) as expp,
                tc.tile_pool(name="gp", bufs=2) as gp,
                tc.tile_pool(name="smallp", bufs=2) as smallp,
                tc.tile_pool(name="otp", bufs=3) as otp,
                tc.tile_pool(name="wop", bufs=6) as wop,
                tc.tile_pool(name="psT", bufs=2, space="PSUM") as psT,
                tc.tile_pool(name="psA", bufs=1, space="PSUM") as psA,
                tc.tile_pool(name="psS", bufs=1, space="PSUM") as psS,
            ):
                wq_sb = wpool.tile([128, KK, 256], BF)
                wk_sb = wpool.tile([128, KK, 256], BF)
                wv_sb = wpool.tile([128, KK, 256], BF)
                nc.sync.dma_start(wq_sb[:], wq[:])

                # ---- Phase 1+2: projections + head-0 attention, interleaved ----
                for b in range(B):
                    for s in range(4):
                        strip = b * 4 + s
                        t0 = strip * 512
                        xq = []
                        for qtr in range(4):
                            xtile = xtp.tile([128, 4, 512], BF, tag="xt")
                            nc.sync.dma_start(
                                xtile[:],
                                xt[:, qtr * 4 : (qtr + 1) * 4, t0 : t0 + 512],
                            )
                            xq.append(xtile)
                        if b == 0 and s == 0:
                            # k/v weights load behind strip-0 x so the q-pass
                            # starts as early as possible
                            nc.sync.dma_start(wk_sb[:], wk[:])
                            nc.sync.dma_start(wv_sb[:], wv[:])

                        # pass A: q for both heads (2 banks)
                        pa = [ps1.tile([128, 512], FP, tag=f"p1{j}", name=f"pa{j}")
                              for j in range(2)]
                        for kk in range(KK):
                            xsl = xq[kk // 4][:, kk % 4, :]
                            st, sp = kk == 0, kk == KK - 1
                            nc.tensor.matmul(pa[0][:], wq_sb[:, kk, 0:128], xsl, start=st, stop=sp)
                            nc.tensor.matmul(pa[1][:], wq_sb[:, kk, 128:256], xsl, start=st, stop=sp)
                        for hl in range(2):
                            nc.scalar.copy(qT[:, hl, t0 : t0 + 512], pa[hl][:])
                        # pass B: k for both heads
                        pb = [ps1.tile([128, 512], FP, tag=f"p1{j}", name=f"pb{j}")
                              for j in range(2)]
                        for kk in range(KK):
                            xsl = xq[kk // 4][:, kk % 4, :]
                            st, sp = kk == 0, kk == KK - 1
                            nc.tensor.matmul(pb[0][:], wk_sb[:, kk, 0:128], xsl, start=st, stop=sp)
                            nc.tensor.matmul(pb[1][:], wk_sb[:, kk, 128:256], xsl, start=st, stop=sp)
                        for hl in range(2):
                            nc.scalar.copy(kT[:, hl, t0 : t0 + 512], pb[hl][:])
                        # pass C/D: v in two tt sub-passes (2 banks each)
                        for half in range(2):
                            pv = [ps1.tile([128, 256], FP, tag=f"p1{j}", name=f"pv{j}")
                                  for j in range(2)]
                            for kk in range(KK):
                                xsl = xq[kk // 4][:, kk % 4, :]
                                st, sp = kk == 0, kk == KK - 1
                                for jj in range(2):
                                    tt = half * 2 + jj
                                    nc.tensor.matmul(
                                        pv[jj][:],
                                        xsl[:, tt * 128 : (tt + 1) * 128],
                                        wv_sb[:, kk, :],
                                        start=st,
                                        stop=sp,
                                    )
                            for jj in range(2):
                                nc.vector.tensor_copy(
                                    v_sb[:, strip * 4 + half * 2 + jj, :], pv[jj][:]
                                )

                        attention_unit(tc, nc, a2a_in, qT, kT, v_sb, ones, masks,
                                       expp, gp, smallp, otp, psT, psA, psS, b, 0, s)

                nc.gpsimd.collective_compute(
                    "AllToAll", mybir.AluOpType.bypass,
                    replica_groups=[list(range(N_CORES))],
                    ins=[a2a_in[0][:].opt()],
                    outs=[a2a_out[0][:].opt()],
                )
                # sync queue is idle during head-1 attention (slot DMAs go via
                # the vector queue), so the a2a-gated wait here blocks nothing
                nc.sync.dma_start(
                    ot_sb[0][:], a2a_out[0][:].rearrange("i f t -> f i t")
                )

                # ---- Phase 3: head-1 attention (covers AllToAll#0) ----
                # biggest units first so the trailing epilogue before the
                # AllToAll#1 kick belongs to the smallest unit
                for s in range(3, -1, -1):
                    for b in range(B):
                        attention_unit(tc, nc, a2a_in, qT, kT, v_sb, ones, masks,
                                       expp, gp, smallp, otp, psT, psA, psS, b, 1, s)

                nc.gpsimd.collective_compute(
                    "AllToAll", mybir.AluOpType.bypass,
                    replica_groups=[list(range(N_CORES))],
                    ins=[a2a_in[1][:].opt()],
                    outs=[a2a_out[1][:].opt()],
                )
                nc.sync.dma_start(
                    ot_sb[1][:], a2a_out[1][:].rearrange("i f t -> f i t")
                )

                # ---- Phase 4: output projection, two passes ----
                # pass 0 (h0 features) overlaps AllToAll#1; pass 1 adds h1.
                for h in range(2):
                    for dd in range(KK):
                        wod = wop.tile([128, 8, 128], BF, tag="wod")
                        nc.sync.dma_start(wod[:], wo[:, h, :, dd * 128 : (dd + 1) * 128])
                        op = ps1.tile([128, 512], FP, tag="p10", name=f"op{h}_{dd}")
                        for i in range(8):
                            nc.tensor.matmul(
                                op[:],
                                wod[:, i, :],
                                ot_sb[h][:, i, :],
                                start=(i == 0),
                                stop=(i == 7),
                            )
                        if h == 0:
                            nc.scalar.copy(acc0[:, dd, :], op[:])
                        else:
                            ob = otp.tile([128, 512], FP, tag="ob")
                            nc.vector.tensor_add(ob[:], op[:], acc0[:, dd, :])
                            nc.sync.dma_start(out_t[dd * 128 : (dd + 1) * 128, :], ob[:])

    nc.compile()
    return nc


def attention_unit(tc, nc, a2a_in, qT, kT, v_sb, ones, masks,
                   expp, gp, smallp, otp, psT, psA, psS, b, hl, s):
    q0 = b * S + s * 512
    qts = qT[:, hl, q0 : q0 + 512]
    avp = psA.tile([128, 512], FP, tag="av")
    smp = psS.tile([1, 512], FP, tag="sm")
    nk = 4 * (s + 1)
    ngroups = s + 1
    for g in range(ngroups):
        diag = g == s
        exs = []
        for j in range(4):
            ki = g * 4 + j
            lo = j * 128 if diag else 0  # causally-active columns start here
            stp = psT.tile([128, 512], FP, tag="st")
            nc.tensor.matmul(
                stp[:, lo:],
                kT[:, hl, b * S + ki * 128 : b * S + (ki + 1) * 128],
                qts[:, lo:],
                start=True,
                stop=True,
            )
            if diag:
                nc.vector.tensor_add(stp[:, lo:], stp[:, lo:], masks[:, j, lo:])
            ex = expp.tile([128, 512], BF, tag="ex")
            nc.scalar.activation(ex[:, lo:], stp[:, lo:], mybir.ActivationFunctionType.Exp)
            nc.tensor.matmul(
                avp[:, lo:],
                v_sb[:, b * 16 + ki, hl * 128 : (hl + 1) * 128],
                ex[:, lo:],
                start=(ki == 0),
                stop=(ki == nk - 1),
                skip_group_check=True,
            )
            exs.append(ex)
        # per-group exp-sum: DVE folds 4 tiles, one ones-matmul per group
        gacc = gp.tile([128, 512], BF, tag="g")
        if diag:
            nc.vector.tensor_copy(gacc[:], exs[0][:])
            for j in range(1, 4):
                lo = j * 128
                nc.vector.tensor_add(gacc[:, lo:], gacc[:, lo:], exs[j][:, lo:])
        else:
            t01 = gp.tile([128, 512], BF, tag="t01")
            nc.vector.tensor_add(t01[:], exs[0][:], exs[1][:])
            nc.vector.tensor_add(gacc[:], exs[2][:], exs[3][:])
            nc.vector.tensor_add(gacc[:], gacc[:], t01[:])
        nc.tensor.matmul(smp[:], ones[:], gacc[:],
                         start=(g == 0), stop=(g == ngroups - 1))
    sums_sb = smallp.tile([1, 512], FP, tag="sums")
    nc.scalar.copy(sums_sb[:], smp[:])
    sbc = smallp.tile([128, 512], FP, tag="sbc")
    nc.gpsimd.partition_broadcast(sbc[:], sums_sb[:])
    rbc = smallp.tile([128, 512], FP, tag="rbc")
    nc.vector.reciprocal_approx_fast(rbc[:], sbc[:])
    ot = otp.tile([128, 512], BF, tag="ot")
    nc.vector.tensor_mul(ot[:], avp[:], rbc[:])
    j = b * 4 + s
    # gpsimd queue: follows this unit's broadcast, so this trigger never
    # blocks anything else and the sync queue stays free for a2a-gated DMAs
    nc.gpsimd.dma_start(a2a_in[hl][j, :, :], ot[:])


_NC_CACHE = {}


def _get_nc():
    if "nc" not in _NC_CACHE:
        _NC_CACHE["nc"] = build_nc()
    return _NC_CACHE["nc"]


def _make_in_maps(x, wq, wk, wv, wo):
    x = np.ascontiguousarray(np.asarray(x, dtype=np.float32))
    wq = np.asarray(wq, dtype=np.float32)
    wk = np.asarray(wk, dtype=np.float32)
    wv = np.asarray(wv, dtype=np.float32)
    wo = np.asarray(wo, dtype=np.float32)

    x_flat = x.reshape(T, D)
    # xt[p, kk, t] = x_flat[t, kk*128+p]
    xt_host = np.ascontiguousarray(
        x_flat.T.reshape(KK, 128, T).transpose(1, 0, 2)
    ).astype(NPBF)
    # wo_dev[p, h, i, d] = wo[d, (2i+h)*128+p]
    wo_host = np.ascontiguousarray(
        wo.T.reshape(8, 2, 128, D).transpose(2, 1, 0, 3)
    ).astype(NPBF)
    scale = 1.0 / np.sqrt(np.float32(DH))

    in_maps = []
    for c in range(N_CORES):
        sl = slice(c * 256, (c + 1) * 256)

        def wslice(w, scaled=False):
            wc = w[sl, :].T  # [D, 256]
            if scaled:
                wc = wc * scale
            return np.ascontiguousarray(
                wc.reshape(KK, 128, 256).transpose(1, 0, 2)
            ).astype(NPBF)

        in_maps.append(
            {
                "xt": xt_host,
                "wq": wslice(wq, scaled=True),
                "wk": wslice(wk),
                "wv": wslice(wv),
                "wo": wo_host,
            }
        )
    return in_maps


def _run(x, wq, wk, wv, wo, trace=False):
    nc = _get_nc()
    in_maps = _make_in_maps(x, wq, wk, wv, wo)
    res = run_bass_kernel_spmd(nc, in_maps, list(range(N_CORES)), trace=trace)
    rows = [res.results[c]["out_t"].T for c in range(N_CORES)]  # [512, D] each
    out = np.concatenate(rows, axis=0).reshape(B, S, D)
    return out, res


def kernel(x, wq, wk, wv, wo):
    out, _ = _run(x, wq, wk, wv, wo, trace=False)
    return out


# revision 17
# speedup vs baseline: 1.0474x; 1.0083x over previous
"""Context-parallel causal attention block on 8 Trainium2 NeuronCores.

Strategy: tensor-parallel split-heads. Each core c computes Q/K/V projections
for its 2 heads (of 16) over all tokens with host-sliced weights, runs causal
attention locally (feature-major layouts, no transposes), then one on-device
AllToAll re-shards from head-parallel to token-parallel, and each core runs the
output projection for its 512-token row slice. Host concatenates row slices.

Schedule: projections interleave with head-0 attention only; AllToAll#0 fires
right after the last head-0 unit and is covered by head-1 attention; AllToAll#1
is covered by the h0-feature half of the output projection (2-pass oproj with
an fp32 SBUF accumulator). Softmax denominators come from a DVE fp32 running
sum of the exp tiles finished by one gpsimd partition_all_reduce per unit (no
tensor-engine ones-matmuls); diagonal blocks stream only their causally-active
columns. Slot DMAs ride the gpsimd queue and a2a-gated rearranges the sync
queue so no engine queue ever blocks on a collective.

Matmul operands are fp16 (same PE rate as bf16, 8x finer mantissa; fp32
matmuls are two-pass / half-rate on TRN2's PE); accumulation stays fp32 in
PSUM, softmax runs on fp32 scores.
"""
import sys

sys.path.insert(0, "/opt/trn_rl_repo")

import numpy as np

import concourse.bass as bass
import concourse.bass_isa as bass_isa
import concourse.tile as tile
from concourse import bacc, mybir
from concourse.bass_utils import run_bass_kernel_spmd

FP = mybir.dt.float32
F16 = mybir.dt.float16
NPF16 = np.float16
N_CORES = 8
B, S, D, H, DH = 2, 2048, 2048, 16, 128
T = B * S            # 4096 flattened tokens, b-major
KK = D // 128        # 16 contraction k-tiles
NSTRIP = T // 512    # 8 token strips of 512
ROWS = T // N_CORES  # 512 output rows per core
NEG = -1.0e30


def build_nc() -> bacc.Bacc:
    nc = bacc.Bacc("TRN2", target_bir_lowering=False, debug=False, num_devices=N_CORES)

    xt = nc.dram_tensor("xt", [128, KK, T], F16, kind="ExternalInput")
    wq = nc.dram_tensor("wq", [128, KK, 256], F16, kind="ExternalInput")
    wk = nc.dram_tensor("wk", [128, KK, 256], F16, kind="ExternalInput")
    wv = nc.dram_tensor("wv", [128, KK, 256], F16, kind="ExternalInput")
    # wo laid out host-side as [p, h, i, d]: feature tile (2i+h), out col d
    wo = nc.dram_tensor("wo", [128, 2, 8, D], F16, kind="ExternalInput")
    out_t = nc.dram_tensor("out_t", [D, ROWS], FP, kind="ExternalOutput")

    with tile.TileContext(nc) as tc:
        with (
            tc.tile_pool(name="dram", bufs=1, space="DRAM") as dram,
            tc.tile_pool(name="consts", bufs=1) as consts,
            tc.tile_pool(name="persist", bufs=1) as persist,
        ):
            a2a_in = [dram.tile([N_CORES, 128, 512], F16, name=f"a2a_in{h}") for h in range(2)]
            a2a_out = [dram.tile([N_CORES, 128, 512], F16, name=f"a2a_out{h}") for h in range(2)]
            wu_in = dram.tile([N_CORES, 1, 8], F16, name="wu_in")
            wu_out = dram.tile([N_CORES, 1, 8], F16, name="wu_out")

            wu_sb = consts.tile([1, 64], F16)
            nc.gpsimd.memset(wu_sb[:], 0.0)
            nc.sync.dma_start(wu_in[:].rearrange("i o t -> o (i t)"), wu_sb[:])
            # warm up the collective stream so the real AllToAlls start fast
            nc.gpsimd.collective_compute(
                "AllToAll", mybir.AluOpType.bypass,
                replica_groups=[list(range(N_CORES))],
                ins=[wu_in[:].opt()],
                outs=[wu_out[:].opt()],
            )
            # additive causal masks for the 4 diagonal offsets:
            # masks[p, i, q] = 0 if q >= p + i*128 else NEG
            masks = consts.tile([128, 4, 512], FP)
            nc.gpsimd.memset(masks[:], 0.0)
            for i in range(4):
                nc.gpsimd.affine_select(
                    out=masks[:, i, :],
                    in_=masks[:, i, :],
                    compare_op=mybir.AluOpType.is_ge,
                    fill=NEG,
                    base=-(i * 128),
                    pattern=[[1, 512]],
                    channel_multiplier=-1,
                )

            qT = persist.tile([128, 2, T], F16)       # [dh, hl, t]
            kT = persist.tile([128, 2, T], F16)
            v_sb = persist.tile([128, 32, 256], F16)  # [t%128, t//128, head_feat]
            acc0 = persist.tile([128, KK, 512], FP)   # oproj h0-pass partials
            ot_sb = [persist.tile([128, 8, 512], F16, name=f"ot_sb{h}") for h in range(2)]

            with (
                tc.tile_pool(name="wpool", bufs=1) as wpool,
                tc.tile_pool(name="xtp", bufs=6) as xtp,
                tc.tile_pool(name="ps1", bufs=2, space="PSUM") as ps1,
                tc.tile_pool(name="expp", bufs=5) as expp,
                tc.tile_pool(name="sap", bufs=2) as sap,
                tc.tile_pool(name="smallp", bufs=2) as smallp,
                tc.tile_pool(name="otp", bufs=3) as otp,
                tc.tile_pool(name="wop", bufs=6) as wop,
                tc.tile_pool(name="psT", bufs=2, space="PSUM") as psT,
                tc.tile_pool(name="psA", bufs=2, space="PSUM") as psA,
            ):
                wq_sb = wpool.tile([128, KK, 256], F16)
                wk_sb = wpool.tile([128, KK, 256], F16)
                wv_sb = wpool.tile([128, KK, 256], F16)
                # first q-pass needs only the first kk tiles: split the load
                nc.sync.dma_start(wq_sb[:, 0:4, :], wq[:, 0:4, :])
                nc.sync.dma_start(wq_sb[:, 4:KK, :], wq[:, 4:KK, :])

                # ---- Phase 1+2: projections + head-0 attention, interleaved ----
                for b in range(B):
                    for s in range(4):
                        strip = b * 4 + s
                        t0 = strip * 512
                        xq = []
                        for qtr in range(4):
                            xtile = xtp.tile([128, 4, 512], F16, tag="xt")
                            nc.sync.dma_start(
                                xtile[:],
                                xt[:, qtr * 4 : (qtr + 1) * 4, t0 : t0 + 512],
                            )
                            xq.append(xtile)
                            if b == 0 and s == 0 and qtr == 0:
                                # k weights right behind the first x quarter so
                                # the k-pass never waits; v behind quarter 1
                                nc.sync.dma_start(wk_sb[:], wk[:])
                            if b == 0 and s == 0 and qtr == 1:
                                nc.sync.dma_start(wv_sb[:], wv[:])

                        # pass A: q for both heads (2 banks)
                        pa = [ps1.tile([128, 512], FP, tag=f"p1{j}", name=f"pa{j}")
                              for j in range(2)]
                        for kk in range(KK):
                            xsl = xq[kk // 4][:, kk % 4, :]
                            st, sp = kk == 0, kk == KK - 1
                            nc.tensor.matmul(pa[0][:], wq_sb[:, kk, 0:128], xsl, start=st, stop=sp)
                            nc.tensor.matmul(pa[1][:], wq_sb[:, kk, 128:256], xsl, start=st, stop=sp)
                        for hl in range(2):
                            nc.scalar.copy(qT[:, hl, t0 : t0 + 512], pa[hl][:])
                        # pass B: k for both heads
                        pb = [ps1.tile([128, 512], FP, tag=f"p1{j}", name=f"pb{j}")
                              for j in range(2)]
                        for kk in range(KK):
                            xsl = xq[kk // 4][:, kk % 4, :]
                            st, sp = kk == 0, kk == KK - 1
                            nc.tensor.matmul(pb[0][:], wk_sb[:, kk, 0:128], xsl, start=st, stop=sp)
                            nc.tensor.matmul(pb[1][:], wk_sb[:, kk, 128:256], xsl, start=st, stop=sp)
                        for hl in range(2):
                            nc.scalar.copy(kT[:, hl, t0 : t0 + 512], pb[hl][:])
                        # pass C/D: v in two tt sub-passes (2 banks each)
                        for half in range(2):
                            pv = [ps1.tile([128, 256], FP, tag=f"p1{j}", name=f"pv{j}")
                                  for j in range(2)]
                            for kk in range(KK):
                                xsl = xq[kk // 4][:, kk % 4, :]
                                st, sp = kk == 0, kk == KK - 1
                                for jj in range(2):
                                    tt = half * 2 + jj
                                    nc.tensor.matmul(
                                        pv[jj][:],
                                        xsl[:, tt * 128 : (tt + 1) * 128],
                                        wv_sb[:, kk, :],
                                        start=st,
                                        stop=sp,
                                    )
                            for jj in range(2):
                                nc.vector.tensor_copy(
                                    v_sb[:, strip * 4 + half * 2 + jj, :], pv[jj][:]
                                )

                        attention_unit(tc, nc, a2a_in, qT, kT, v_sb, masks,
                                       expp, sap, smallp, otp, psT, psA, b, 0, s)

                nc.gpsimd.collective_compute(
                    "AllToAll", mybir.AluOpType.bypass,
                    replica_groups=[list(range(N_CORES))],
                    ins=[a2a_in[0][:].opt()],
                    outs=[a2a_out[0][:].opt()],
                )
                # sync queue is idle during head-1 attention (slot DMAs go via
                # the gpsimd queue), so the a2a-gated waits here block nothing;
                # two halves let downstream consumers start sooner
                for ih in range(2):
                    nc.sync.dma_start(
                        ot_sb[0][:, 4 * ih : 4 * ih + 4, :],
                        a2a_out[0][4 * ih : 4 * ih + 4, :, :].rearrange("i f t -> f i t"),
                    )

                # ---- Phase 3: head-1 attention (covers AllToAll#0) ----
                # biggest units first so the trailing epilogue before the
                # AllToAll#1 kick belongs to the smallest unit
                for s in range(3, -1, -1):
                    for b in range(B):
                        attention_unit(tc, nc, a2a_in, qT, kT, v_sb, masks,
                                       expp, sap, smallp, otp, psT, psA, b, 1, s)

                nc.gpsimd.collective_compute(
                    "AllToAll", mybir.AluOpType.bypass,
                    replica_groups=[list(range(N_CORES))],
                    ins=[a2a_in[1][:].opt()],
                    outs=[a2a_out[1][:].opt()],
                )
                for ih in range(2):
                    nc.sync.dma_start(
                        ot_sb[1][:, 4 * ih : 4 * ih + 4, :],
                        a2a_out[1][4 * ih : 4 * ih + 4, :, :].rearrange("i f t -> f i t"),
                    )

                # ---- Phase 4: output projection, two passes ----
                # pass 0 (h0 features) overlaps AllToAll#1; pass 1 adds h1.
                for h in range(2):
                    for dd in range(KK):
                        wod = wop.tile([128, 8, 128], F16, tag="wod")
                        nc.sync.dma_start(wod[:], wo[:, h, :, dd * 128 : (dd + 1) * 128])
                        op = ps1.tile([128, 512], FP, tag="p10", name=f"op{h}_{dd}")
                        for i in range(8):
                            nc.tensor.matmul(
                                op[:],
                                wod[:, i, :],
                                ot_sb[h][:, i, :],
                                start=(i == 0),
                                stop=(i == 7),
                            )
                        if h == 0:
                            nc.scalar.copy(acc0[:, dd, :], op[:])
                        else:
                            ob = otp.tile([128, 512], FP, tag="ob")
                            nc.vector.tensor_add(ob[:], op[:], acc0[:, dd, :])
                            nc.sync.dma_start(out_t[dd * 128 : (dd + 1) * 128, :], ob[:])

    nc.compile()
    return nc


def attention_unit(tc, nc, a2a_in, qT, kT, v_sb, masks,
                   expp, sap, smallp, otp, psT, psA, b, hl, s):
    q0 = b * S + s * 512
    qts = qT[:, hl, q0 : q0 + 512]
    avp = psA.tile([128, 512], FP, tag="av")
    nk = 4 * (s + 1)
    sacc = sap.tile([128, 512], FP, tag="sa")  # fp32 running exp-sum
    for ki in range(nk):
        j = ki % 4
        diag = ki // 4 == s
        lo = j * 128 if diag else 0  # causally-active columns start here
        stp = psT.tile([128, 512], FP, tag="st")
        nc.tensor.matmul(
            stp[:, lo:],
            kT[:, hl, b * S + ki * 128 : b * S + (ki + 1) * 128],
            qts[:, lo:],
            start=True,
            stop=True,
        )
        if diag:
            nc.vector.tensor_add(stp[:, lo:], stp[:, lo:], masks[:, j, lo:])
        ex = expp.tile([128, 512], F16, tag="ex")
        nc.scalar.activation(ex[:, lo:], stp[:, lo:], mybir.ActivationFunctionType.Exp)
        nc.tensor.matmul(
            avp[:, lo:],
            v_sb[:, b * 16 + ki, hl * 128 : (hl + 1) * 128],
            ex[:, lo:],
            start=(ki == 0),
            stop=(ki == nk - 1),
            skip_group_check=True,
        )
        if ki == 0:
            nc.vector.tensor_copy(sacc[:], ex[:])
        else:
            nc.vector.tensor_add(sacc[:, lo:], sacc[:, lo:], ex[:, lo:])
    # softmax denominator: all-reduce the fp32 partials across partitions
    sbc = smallp.tile([128, 512], FP, tag="sbc")
    nc.gpsimd.partition_all_reduce(sbc[:], sacc[:], 128, bass_isa.ReduceOp.add)
    rbc = smallp.tile([128, 512], FP, tag="rbc")
    nc.vector.reciprocal_approx_fast(rbc[:], sbc[:])
    ot = otp.tile([128, 512], F16, tag="ot")
    nc.vector.tensor_mul(ot[:], avp[:], rbc[:])
    j = b * 4 + s
    # gpsimd queue: follows this unit's all-reduce, so this trigger never
    # blocks anything else and the sync queue stays free for a2a-gated DMAs
    nc.gpsimd.dma_start(a2a_in[hl][j, :, :], ot[:])


_NC_CACHE = {}


def _get_nc():
    if "nc" not in _NC_CACHE:
        _NC_CACHE["nc"] = build_nc()
    return _NC_CACHE["nc"]


def _make_in_maps(x, wq, wk, wv, wo):
    x = np.ascontiguousarray(np.asarray(x, dtype=np.float32))
    wq = np.asarray(wq, dtype=np.float32)
    wk = np.asarray(wk, dtype=np.float32)
    wv = np.asarray(wv, dtype=np.float32)
    wo = np.asarray(wo, dtype=np.float32)

    x_flat = x.reshape(T, D)
    # xt[p, kk, t] = x_flat[t, kk*128+p]
    xt_host = np.ascontiguousarray(
        x_flat.T.reshape(KK, 128, T).transpose(1, 0, 2)
    ).astype(NPF16)
    # wo_dev[p, h, i, d] = wo[d, (2i+h)*128+p]
    wo_host = np.ascontiguousarray(
        wo.T.reshape(8, 2, 128, D).transpose(2, 1, 0, 3)
    ).astype(NPF16)
    scale = 1.0 / np.sqrt(np.float32(DH))

    in_maps = []
    for c in range(N_CORES):
        sl = slice(c * 256, (c + 1) * 256)

        def wslice(w, scaled=False):
            wc = w[sl, :].T  # [D, 256]
            if scaled:
                wc = wc * scale
            return np.ascontiguousarray(
                wc.reshape(KK, 128, 256).transpose(1, 0, 2)
            ).astype(NPF16)

        in_maps.append(
            {
                "xt": xt_host,
                "wq": wslice(wq, scaled=True),
                "wk": wslice(wk),
                "wv": wslice(wv),
                "wo": wo_host,
            }
        )
    return in_maps


def _run(x, wq, wk, wv, wo, trace=False):
    nc = _get_nc()
    in_maps = _make_in_maps(x, wq, wk, wv, wo)
    res = run_bass_kernel_spmd(nc, in_maps, list(range(N_CORES)), trace=trace)
    rows = [res.results[c]["out_t"].T for c in range(N_CORES)]  # [512, D] each
    out = np.concatenate(rows, axis=0).reshape(B, S, D)
    return out, res


def kernel(x, wq, wk, wv, wo):
    out, _ = _run(x, wq, wk, wv, wo, trace=False)
    return out
